# revision 12
# baseline (speedup 1.0000x reference)
"""Trainium2 Bass kernel for nn_MultiHeadAttention_8400956031164.

Full attention block: QKV proj + per-head RMSNorm + RoPE + sliding-window
causal attention (WIN=1024) + output proj.

Sharding: tensor-parallel over heads across 8 cores (2 heads/core), both
batches looped per core. Host sums the 8 partial Wo outputs.

Device-side layout strategy (per core):
  - X^T [D, S] streamed per 512-token group; Q,K produced TRANSPOSED
    [hd=128, s] per head directly from projection (lhsT = W slice).
  - RMSNorm in transposed layout: sum(q^2) over hd via all-ones matmul
    (result broadcast across partitions in PSUM), sqrt on ACT, reciprocal
    on DVE, multiply. q_scale/k_scale/softmax SCALE folded into RoPE
    cos/sin tables on host.
  - RoPE in transposed layout: rotate_half = partition-half swap; sign
    folded into host-precomputed sin tables.
  - Scores computed transposed S^T[k, q] (k on partitions) per 128x(<=512)
    block over the sliding window; exp on ACT (PSUM->SBUF, f32r out);
    causal/window triangle masks applied by DVE mask-multiply.
  - Softmax denominator via all-ones matmul accumulation (broadcast sums);
    PV accumulates V^T @ P^T = out^T [hd, q] in PSUM with variable-N
    has_written semantics; normalization folded as out^T * (1/L).
  - Wo: lhsT = normalized out^T slices, accumulate 2 head-chunks, ACT
    copy PSUM->SBUF, DMA out.
  All matmuls in float32r (TF32-class, ~1.5e-4 relerr, full PE rate).
"""

import numpy as np
from contextlib import ExitStack

import concourse.bass as bass
import concourse.tile as tile
import concourse.mybir as mybir
from concourse import bacc, bass_utils

B, S, D, H, HD, WIN = 2, 2048, 2048, 16, 128, 1024
EPS = 1e-6
SCALE = HD ** -0.5
NCORES = 8
HLOC = H // NCORES          # heads per core = 2
NL = HLOC * HD              # local head dims = 256
SG = 512                    # token group size
G = S // SG                 # groups per batch = 4
NDK = D // 128              # contraction chunks = 16

F32 = mybir.dt.float32
F32R = mybir.dt.float32r
AF = mybir.ActivationFunctionType

_CACHE = {}


def _build():
    nc = bacc.Bacc(trn_type="TRN2", target_bir_lowering=False, debug=False)

    def din(name, shape, dt):
        return nc.dram_tensor(name, shape, dt, kind="ExternalInput").ap()

    xt = din("xt", [D, B * S], F32R)
    wq = din("wq", [D, NL], F32R)
    wk = din("wk", [D, NL], F32R)
    wv = din("wv", [D, NL], F32R)
    wo = din("wo", [NL, D], F32R)
    cq = din("cq", [HD, B * S], F32)   # cos * q_scale * SCALE, transposed
    sq = din("sq", [HD, B * S], F32)   # sin * roll(q_scale) * SCALE, lower half negated
    ck = din("ck", [HD, B * S], F32)
    sk = din("sk", [HD, B * S], F32)
    ones_d = din("ones_d", [128, 128], F32R)
    rperm_d = din("rperm_d", [128, 128], F32R)  # rotate_half as matmul, signs included
    mdiag_d = din("mdiag_d", [128, 128], F32)   # keep kk <= qq
    medge_d = din("medge_d", [128, 128], F32)   # keep kk >= qq
    opart = nc.dram_tensor("opart", [B * S, D], F32, kind="ExternalOutput").ap()

    xtr = xt.rearrange("(a p) s -> p a s", p=128)     # [128, 16, B*S]
    wqr = wq.rearrange("(a p) n -> p a n", p=128)     # [128, 16, NL]
    wkr = wk.rearrange("(a p) n -> p a n", p=128)
    wvr = wv.rearrange("(a p) n -> p a n", p=128)
    wor = wo.rearrange("(c p) d -> p c d", p=128)     # [128, 2, D]

    with tile.TileContext(nc) as tc, ExitStack() as ctx:
        const = ctx.enter_context(tc.tile_pool(name="const", bufs=1))
        wpool = ctx.enter_context(tc.tile_pool(name="w", bufs=1))
        xpool = ctx.enter_context(tc.tile_pool(name="x", bufs=2))
        cspool = ctx.enter_context(tc.tile_pool(name="cs", bufs=2))
        qpool = ctx.enter_context(tc.tile_pool(name="qr", bufs=3))
        kpool = ctx.enter_context(tc.tile_pool(name="kr", bufs=10))
        vpool = ctx.enter_context(tc.tile_pool(name="v", bufs=18))
        rpool = ctx.enter_context(tc.tile_pool(name="rms", bufs=2))
        ppool = ctx.enter_context(tc.tile_pool(name="p", bufs=6))
        lpool = ctx.enter_context(tc.tile_pool(name="lin", bufs=2))
        opool = ctx.enter_context(tc.tile_pool(name="osb", bufs=3))
        outp = ctx.enter_context(tc.tile_pool(name="out", bufs=3))
        psA = ctx.enter_context(tc.tile_pool(name="psA", bufs=3, space="PSUM"))
        psS = ctx.enter_context(tc.tile_pool(name="psS", bufs=3, space="PSUM"))
        psO = ctx.enter_context(tc.tile_pool(name="psO", bufs=1, space="PSUM"))
        psL = ctx.enter_context(tc.tile_pool(name="psL", bufs=1, space="PSUM"))

        ones_t = const.tile([128, 128], F32R, tag="ones")
        nc.sync.dma_start(ones_t[:], ones_d)
        rperm_t = const.tile([128, 128], F32R, tag="rperm")
        nc.sync.dma_start(rperm_t[:], rperm_d)
        md_t = const.tile([128, 128], F32, tag="md")
        nc.sync.dma_start(md_t[:], mdiag_d)
        me_t = const.tile([128, 128], F32, tag="me")
        nc.sync.dma_start(me_t[:], medge_d)
        eps_t = const.tile([128, 1], F32, tag="eps")
        nc.vector.memset(eps_t[:], EPS)

        wq_t = wpool.tile([128, NDK, NL], F32R, tag="wq")
        nc.sync.dma_start(wq_t[:], wqr)
        wk_t = wpool.tile([128, NDK, NL], F32R, tag="wk")
        nc.sync.dma_start(wk_t[:], wkr)
        wv_t = wpool.tile([128, NDK, NL], F32R, tag="wv")
        nc.sync.dma_start(wv_t[:], wvr)
        wo_t = wpool.tile([128, HLOC, D], F32R, tag="wo")
        nc.sync.dma_start(wo_t[:], wor)

        for b in range(B):
            KrT = {}   # (h, g) -> [128, SG] f32r tile
            Vt = {}    # s-tile index -> [128, NL] f32r tile
            for g in range(G):
                s0 = b * S + g * SG

                # ---- input streams for this group ----
                xh0 = xpool.tile([128, 8, SG], F32R, tag="xt")
                nc.sync.dma_start(xh0[:], xtr[:, 0:8, s0:s0 + SG])
                xh1 = xpool.tile([128, 8, SG], F32R, tag="xt")
                nc.sync.dma_start(xh1[:], xtr[:, 8:16, s0:s0 + SG])

                def xs(dk):
                    t = xh0 if dk < 8 else xh1
                    return t[:, dk % 8, :]

                cqt = cspool.tile([128, SG], F32, tag="cq")
                nc.sync.dma_start(cqt[:], cq[:, s0:s0 + SG])
                sqt = cspool.tile([128, SG], F32, tag="sq")
                nc.sync.dma_start(sqt[:], sq[:, s0:s0 + SG])
                ckt = cspool.tile([128, SG], F32, tag="ck")
                nc.sync.dma_start(ckt[:], ck[:, s0:s0 + SG])
                skt = cspool.tile([128, SG], F32, tag="sk")
                nc.sync.dma_start(skt[:], sk[:, s0:s0 + SG])

                # ---- Q/K transposed projections + RMSNorm + RoPE ----
                # PE pipelining: after each projection chain, emit the
                # previous chain's sum-of-squares matmul and the chain
                # before that's rotate matmul, so PE never waits on ACT/DVE.
                qr_tiles = {}

                states = []

                def emit_ss(stt):
                    ssps = psS.tile([128, SG], F32, tag="score")
                    nc.tensor.matmul(ssps[:], ones_t[:], stt["qsq"][:],
                                     start=True, stop=True)
                    rstd = rpool.tile([128, SG], F32, tag="rstd")
                    nc.scalar.activation(rstd[:], ssps[:], AF.Sqrt,
                                         bias=eps_t[:, 0:1], scale=1.0 / HD)
                    nc.vector.reciprocal(rstd[:], rstd[:])
                    qn = rpool.tile([128, SG], F32R, tag="qn")
                    nc.vector.tensor_mul(qn[:], stt["ps"][:], rstd[:])
                    t1 = rpool.tile([128, SG], F32, tag="t1")
                    cost = cqt if stt["t"] == "q" else ckt
                    nc.vector.tensor_mul(t1[:], qn[:], cost[:])
                    stt["qn"] = qn
                    stt["t1"] = t1

                def emit_rot(stt):
                    rot_ps = psS.tile([128, SG], F32, tag="score")
                    nc.tensor.matmul(rot_ps[:], rperm_t[:], stt["qn"][:],
                                     start=True, stop=True)
                    sint = sqt if stt["t"] == "q" else skt
                    dst = stt["dst"]
                    nc.vector.tensor_mul(dst[:], rot_ps[:], sint[:])
                    nc.vector.tensor_add(dst[:], dst[:], stt["t1"][:])

                chains = [("q", 0), ("k", 0), ("q", 1), ("k", 1),
                          ("v", 0), ("v", 1), ("v", 2), ("v", 3)]
                for i, (t, h) in enumerate(chains):
                    if t in ("q", "k"):
                        w_t = wq_t if t == "q" else wk_t
                        ps = psA.tile([128, SG], F32, tag="a")
                        for dk in range(NDK):
                            nc.tensor.matmul(
                                ps[:], w_t[:, dk, h * HD:(h + 1) * HD], xs(dk),
                                start=(dk == 0), stop=(dk == NDK - 1))
                        qsq = rpool.tile([128, SG], F32R, tag="qsq")
                        nc.scalar.activation(qsq[:], ps[:], AF.Square)
                        if t == "q":
                            dst = qpool.tile([128, SG], F32R, tag="qr")
                            qr_tiles[h] = dst
                        else:
                            dst = kpool.tile([128, SG], F32R, tag="kr")
                            KrT[(h, g)] = dst
                        states.append({"ps": ps, "qsq": qsq, "t": t, "dst": dst})
                    else:
                        st = h
                        psv = psA.tile([128, NL], F32, tag="a")
                        for dk in range(NDK):
                            nc.tensor.matmul(
                                psv[:], xs(dk)[:, st * 128:(st + 1) * 128],
                                wv_t[:, dk, :],
                                start=(dk == 0), stop=(dk == NDK - 1))
                        vt = vpool.tile([128, NL], F32R, tag="v")
                        nc.scalar.copy(vt[:], psv[:])
                        Vt[4 * g + st] = vt
                    if 0 <= i - 1 < 4:
                        emit_ss(states[i - 1])
                    if 0 <= i - 2 < 4:
                        emit_rot(states[i - 2])

                # ---- attention for each head ----
                osbs = {}
                for h in range(HLOC):
                    qr_t = qr_tiles[h]
                    kts = list(range(max(0, 4 * g - 8), 4 * g + 4))
                    oacc = psO.tile([128, SG], F32, tag="o")
                    lacc = psL.tile([128, SG], F32, tag="l")

                    pend = []

                    def emit_pv(item, first, last):
                        kt, qoff, n, p = item
                        nc.tensor.matmul(
                            oacc[:, qoff:qoff + n],
                            Vt[kt][:, h * HD:(h + 1) * HD], p[:],
                            start=first, stop=last)
                        nc.tensor.matmul(
                            lacc[:, qoff:qoff + n], ones_t[:], p[:],
                            start=first, stop=last)

                    LAG = 3
                    for i, kt in enumerate(kts):
                        qt_lo = max(4 * g, kt)
                        qt_hi = min(4 * g + 3, kt + 8)
                        qoff = 128 * (qt_lo - 4 * g)
                        n = 128 * (qt_hi - qt_lo + 1)
                        sc = psS.tile([128, n], F32, tag="score")
                        kr_t = KrT[(h, kt // 4)]
                        c = (kt % 4) * 128
                        nc.tensor.matmul(sc[:], kr_t[:, c:c + 128],
                                         qr_t[:, qoff:qoff + n],
                                         start=True, stop=True)
                        p = ppool.tile([128, n], F32R, tag="p")
                        nc.scalar.activation(p[:], sc[:], AF.Exp)
                        if kt >= 4 * g:
                            nc.vector.tensor_mul(p[:, 0:128], p[:, 0:128], md_t[:])
                        if kt + 8 <= 4 * g + 3:
                            nc.vector.tensor_mul(p[:, n - 128:n], p[:, n - 128:n],
                                                 me_t[:])
                        pend.append((kt, qoff, n, p))
                        if i >= LAG:
                            emit_pv(pend[i - LAG], first=(i - LAG == 0), last=False)
                    nk = len(kts)
                    for j in range(max(0, nk - LAG), nk):
                        emit_pv(pend[j], first=(j == 0), last=(j == nk - 1))

                    linv = lpool.tile([128, SG], F32, tag="lin")
                    nc.vector.reciprocal(linv[:], lacc[:])
                    osb = opool.tile([128, SG], F32R, tag="osb")
                    nc.vector.tensor_mul(osb[:], oacc[:], linv[:])
                    osbs[h] = osb

                # ---- Wo partial for this group ----
                for st in range(4):
                    for dg in range(4):
                        pso = psA.tile([128, 512], F32, tag="a")
                        nc.tensor.matmul(pso[:],
                                         osbs[0][:, st * 128:(st + 1) * 128],
                                         wo_t[:, 0, dg * 512:(dg + 1) * 512],
                                         start=True, stop=False)
                        nc.tensor.matmul(pso[:],
                                         osbs[1][:, st * 128:(st + 1) * 128],
                                         wo_t[:, 1, dg * 512:(dg + 1) * 512],
                                         start=False, stop=True)
                        ot = outp.tile([128, 512], F32, tag="out")
                        nc.scalar.copy(ot[:], pso[:])
                        row = s0 + st * 128
                        nc.sync.dma_start(
                            opart[row:row + 128, dg * 512:(dg + 1) * 512], ot[:])

    nc.compile()
    return nc


def _host_prep(hidden_states, cos, sin, Wq, Wk, Wv, Wo, q_scale, k_scale):
    f32 = np.float32
    hs = np.asarray(hidden_states, f32)
    cos = np.asarray(cos, f32)
    sin = np.asarray(sin, f32)
    qs = np.asarray(q_scale, f32)
    ks = np.asarray(k_scale, f32)

    xt = np.concatenate([hs[b].T for b in range(B)], axis=1)  # [D, B*S]
    xt = np.ascontiguousarray(xt, f32)

    qs_rot = np.roll(qs, -64)
    ks_rot = np.roll(ks, -64)

    def rope_tables(scale_vec, rot_vec, extra):
        c = np.concatenate([(cos[b] * scale_vec[None, :] * extra).T
                            for b in range(B)], axis=1)
        s = np.concatenate([(sin[b] * rot_vec[None, :] * extra).T
                            for b in range(B)], axis=1)
        return (np.ascontiguousarray(c, f32), np.ascontiguousarray(s, f32))

    cqa, sqa = rope_tables(qs, qs_rot, SCALE)
    cka, ska = rope_tables(ks, ks_rot, 1.0)

    ones = np.ones((128, 128), f32)
    mdiag = np.triu(np.ones((128, 128), f32))   # keep kk <= qq
    medge = np.tril(np.ones((128, 128), f32))   # keep kk >= qq
    # rotate_half as a matmul: out[p, s] = sum_k rperm[k, p] * x[k, s]
    rperm = np.zeros((128, 128), f32)
    rperm[np.arange(64) + 64, np.arange(64)] = -1.0   # out[p<64] = -x[p+64]
    rperm[np.arange(64), np.arange(64) + 64] = 1.0    # out[p>=64] = x[p-64]

    shared = {
        "xt": xt, "cq": cqa, "sq": sqa, "ck": cka, "sk": ska,
        "ones_d": ones, "mdiag_d": mdiag, "medge_d": medge, "rperm_d": rperm,
    }
    Wq = np.asarray(Wq, f32)
    Wk = np.asarray(Wk, f32)
    Wv = np.asarray(Wv, f32)
    Wo = np.asarray(Wo, f32)
    in_maps = []
    for c in range(NCORES):
        m = dict(shared)
        m["wq"] = np.ascontiguousarray(Wq[:, c * NL:(c + 1) * NL])
        m["wk"] = np.ascontiguousarray(Wk[:, c * NL:(c + 1) * NL])
        m["wv"] = np.ascontiguousarray(Wv[:, c * NL:(c + 1) * NL])
        m["wo"] = np.ascontiguousarray(Wo[c * NL:(c + 1) * NL, :])
        in_maps.append(m)
    return in_maps


def get_nc():
    if "nc" not in _CACHE:
        _CACHE["nc"] = _build()
    return _CACHE["nc"]


def kernel(hidden_states, cos, sin, Wq, Wk, Wv, Wo, q_scale, k_scale):
    nc = get_nc()
    in_maps = _host_prep(hidden_states, cos, sin, Wq, Wk, Wv, Wo,
                         q_scale, k_scale)
    res = bass_utils.run_bass_kernel_spmd(nc, in_maps,
                                          core_ids=list(range(NCORES)))
    acc = np.zeros((B * S, D), np.float64)
    for r in res.results:
        acc += r["opart"].astype(np.float64)
    return np.ascontiguousarray(
        acc.reshape(B, S, D).astype(np.float32))


# revision 15
# speedup vs baseline: 16.8143x; 16.8143x over previous
"""Trainium2 Bass kernel for nn_MultiHeadAttention_8400956031164.

Full attention block: QKV proj + per-head RMSNorm + RoPE + sliding-window
causal attention (WIN=1024) + output proj.

Sharding: tensor-parallel over heads across 8 cores (2 heads/core), both
batches looped per core. Host sums the 8 partial Wo outputs.

Device-side layout strategy (per core):
  - X^T [D, S] streamed per 512-token group; Q,K produced TRANSPOSED
    [hd=128, s] per head directly from projection (lhsT = W slice).
  - RMSNorm in transposed layout: sum(q^2) over hd via all-ones matmul
    (result broadcast across partitions in PSUM), sqrt on ACT, reciprocal
    on DVE, multiply. q_scale/k_scale/softmax SCALE folded into RoPE
    cos/sin tables on host.
  - RoPE in transposed layout: rotate_half = partition-half swap; sign
    folded into host-precomputed sin tables.
  - Scores computed transposed S^T[k, q] (k on partitions) per 128x(<=512)
    block over the sliding window; exp on ACT (PSUM->SBUF, f32r out);
    causal/window triangle masks applied by DVE mask-multiply.
  - Softmax denominator via all-ones matmul accumulation (broadcast sums);
    PV accumulates V^T @ P^T = out^T [hd, q] in PSUM with variable-N
    has_written semantics; normalization folded as out^T * (1/L).
  - Wo: lhsT = normalized out^T slices, accumulate 2 head-chunks, ACT
    copy PSUM->SBUF, DMA out.
  All matmuls in float32r (TF32-class, ~1.5e-4 relerr, full PE rate).
"""

import numpy as np
from contextlib import ExitStack

import concourse.bass as bass
import concourse.tile as tile
import concourse.mybir as mybir
from concourse import bacc, bass_utils

B, S, D, H, HD, WIN = 2, 2048, 2048, 16, 128, 1024
EPS = 1e-6
SCALE = HD ** -0.5
NCORES = 8
HLOC = H // NCORES          # heads per core = 2
NL = HLOC * HD              # local head dims = 256
SG = 512                    # token group size
G = S // SG                 # groups per batch = 4
NDK = D // 128              # contraction chunks = 16

F32 = mybir.dt.float32
F32R = mybir.dt.float32r
AF = mybir.ActivationFunctionType

_CACHE = {}


def _build(repeat=1):
    nc = bacc.Bacc(trn_type="TRN2", target_bir_lowering=False, debug=False)

    def din(name, shape, dt):
        return nc.dram_tensor(name, shape, dt, kind="ExternalInput").ap()

    xt = din("xt", [D, B * S], F32R)
    wq = din("wq", [D, NL], F32R)
    wk = din("wk", [D, NL], F32R)
    wv = din("wv", [D, NL], F32R)
    wo = din("wo", [NL, D], F32R)
    cq = din("cq", [HD, B * S], F32)   # cos * q_scale * SCALE, transposed
    sq = din("sq", [HD, B * S], F32)   # sin * roll(q_scale) * SCALE, lower half negated
    ck = din("ck", [HD, B * S], F32)
    sk = din("sk", [HD, B * S], F32)
    ones_d = din("ones_d", [128, 128], F32R)
    rperm_d = din("rperm_d", [128, 128], F32R)  # rotate_half as matmul, signs included
    mdiag_d = din("mdiag_d", [128, 128], F32)   # keep kk <= qq
    medge_d = din("medge_d", [128, 128], F32)   # keep kk >= qq
    opart = nc.dram_tensor("opart", [B * S, D], F32, kind="ExternalOutput").ap()

    xtr = xt.rearrange("(a p) s -> p a s", p=128)     # [128, 16, B*S]
    wqr = wq.rearrange("(a p) n -> p a n", p=128)     # [128, 16, NL]
    wkr = wk.rearrange("(a p) n -> p a n", p=128)
    wvr = wv.rearrange("(a p) n -> p a n", p=128)
    wor = wo.rearrange("(c p) d -> p c d", p=128)     # [128, 2, D]

    with tile.TileContext(nc) as tc, ExitStack() as ctx:
        const = ctx.enter_context(tc.tile_pool(name="const", bufs=1))
        wpool = ctx.enter_context(tc.tile_pool(name="w", bufs=1))
        xpool = ctx.enter_context(tc.tile_pool(name="x", bufs=2))
        cspool = ctx.enter_context(tc.tile_pool(name="cs", bufs=2))
        qpool = ctx.enter_context(tc.tile_pool(name="qr", bufs=3))
        kpool = ctx.enter_context(tc.tile_pool(name="kr", bufs=10))
        vpool = ctx.enter_context(tc.tile_pool(name="v", bufs=18))
        rpool = ctx.enter_context(tc.tile_pool(name="rms", bufs=2))
        ppool = ctx.enter_context(tc.tile_pool(name="p", bufs=6))
        lpool = ctx.enter_context(tc.tile_pool(name="lin", bufs=2))
        opool = ctx.enter_context(tc.tile_pool(name="osb", bufs=3))
        outp = ctx.enter_context(tc.tile_pool(name="out", bufs=3))
        psA = ctx.enter_context(tc.tile_pool(name="psA", bufs=3, space="PSUM"))
        psS = ctx.enter_context(tc.tile_pool(name="psS", bufs=3, space="PSUM"))
        psO = ctx.enter_context(tc.tile_pool(name="psO", bufs=1, space="PSUM"))
        psL = ctx.enter_context(tc.tile_pool(name="psL", bufs=1, space="PSUM"))

        ones_t = const.tile([128, 128], F32R, tag="ones")
        nc.sync.dma_start(ones_t[:], ones_d)
        rperm_t = const.tile([128, 128], F32R, tag="rperm")
        nc.sync.dma_start(rperm_t[:], rperm_d)
        md_t = const.tile([128, 128], F32, tag="md")
        nc.sync.dma_start(md_t[:], mdiag_d)
        me_t = const.tile([128, 128], F32, tag="me")
        nc.sync.dma_start(me_t[:], medge_d)
        eps_t = const.tile([128, 1], F32, tag="eps")
        nc.vector.memset(eps_t[:], EPS)

        wq_t = wpool.tile([128, NDK, NL], F32R, tag="wq")
        nc.sync.dma_start(wq_t[:], wqr)
        wk_t = wpool.tile([128, NDK, NL], F32R, tag="wk")
        nc.sync.dma_start(wk_t[:], wkr)
        wv_t = wpool.tile([128, NDK, NL], F32R, tag="wv")
        nc.sync.dma_start(wv_t[:], wvr)
        wo_t = wpool.tile([128, HLOC, D], F32R, tag="wo")
        nc.sync.dma_start(wo_t[:], wor)

        for b in [b_ for _ in range(repeat) for b_ in range(B)]:
            KrT = {}   # (h, g) -> [128, SG] f32r tile
            Vt = {}    # s-tile index -> [128, NL] f32r tile
            for g in range(G):
                s0 = b * S + g * SG

                # ---- input streams for this group ----
                xh0 = xpool.tile([128, 8, SG], F32R, tag="xt")
                nc.sync.dma_start(xh0[:], xtr[:, 0:8, s0:s0 + SG])
                xh1 = xpool.tile([128, 8, SG], F32R, tag="xt")
                nc.sync.dma_start(xh1[:], xtr[:, 8:16, s0:s0 + SG])

                def xs(dk):
                    t = xh0 if dk < 8 else xh1
                    return t[:, dk % 8, :]

                cqt = cspool.tile([128, SG], F32, tag="cq")
                nc.sync.dma_start(cqt[:], cq[:, s0:s0 + SG])
                sqt = cspool.tile([128, SG], F32, tag="sq")
                nc.sync.dma_start(sqt[:], sq[:, s0:s0 + SG])
                ckt = cspool.tile([128, SG], F32, tag="ck")
                nc.sync.dma_start(ckt[:], ck[:, s0:s0 + SG])
                skt = cspool.tile([128, SG], F32, tag="sk")
                nc.sync.dma_start(skt[:], sk[:, s0:s0 + SG])

                # ---- Q/K transposed projections + RMSNorm + RoPE ----
                # PE pipelining: after each projection chain, emit the
                # previous chain's sum-of-squares matmul and the chain
                # before that's rotate matmul, so PE never waits on ACT/DVE.
                qr_tiles = {}

                states = []

                def emit_ss(stt):
                    ssps = psS.tile([128, SG], F32, tag="score")
                    nc.tensor.matmul(ssps[:], ones_t[:], stt["qsq"][:],
                                     start=True, stop=True)
                    rstd = rpool.tile([128, SG], F32, tag="rstd")
                    nc.scalar.activation(rstd[:], ssps[:], AF.Sqrt,
                                         bias=eps_t[:, 0:1], scale=1.0 / HD)
                    nc.vector.reciprocal(rstd[:], rstd[:])
                    qn = rpool.tile([128, SG], F32R, tag="qn")
                    nc.vector.tensor_mul(qn[:], stt["ps"][:], rstd[:])
                    t1 = rpool.tile([128, SG], F32, tag="t1")
                    cost = cqt if stt["t"] == "q" else ckt
                    nc.vector.tensor_mul(t1[:], qn[:], cost[:])
                    stt["qn"] = qn
                    stt["t1"] = t1

                def emit_rot(stt):
                    rot_ps = psS.tile([128, SG], F32, tag="score")
                    nc.tensor.matmul(rot_ps[:], rperm_t[:], stt["qn"][:],
                                     start=True, stop=True)
                    sint = sqt if stt["t"] == "q" else skt
                    dst = stt["dst"]
                    nc.vector.tensor_mul(dst[:], rot_ps[:], sint[:])
                    nc.vector.tensor_add(dst[:], dst[:], stt["t1"][:])

                chains = [("q", 0), ("k", 0), ("q", 1), ("k", 1),
                          ("v", 0), ("v", 1), ("v", 2), ("v", 3)]
                for i, (t, h) in enumerate(chains):
                    if t in ("q", "k"):
                        w_t = wq_t if t == "q" else wk_t
                        ps = psA.tile([128, SG], F32, tag="a")
                        for dk in range(NDK):
                            nc.tensor.matmul(
                                ps[:], w_t[:, dk, h * HD:(h + 1) * HD], xs(dk),
                                start=(dk == 0), stop=(dk == NDK - 1))
                        qsq = rpool.tile([128, SG], F32R, tag="qsq")
                        nc.scalar.activation(qsq[:], ps[:], AF.Square)
                        if t == "q":
                            dst = qpool.tile([128, SG], F32R, tag="qr")
                            qr_tiles[h] = dst
                        else:
                            dst = kpool.tile([128, SG], F32R, tag="kr")
                            KrT[(h, g)] = dst
                        states.append({"ps": ps, "qsq": qsq, "t": t, "dst": dst})
                    else:
                        st = h
                        psv = psA.tile([128, NL], F32, tag="a")
                        for dk in range(NDK):
                            nc.tensor.matmul(
                                psv[:], xs(dk)[:, st * 128:(st + 1) * 128],
                                wv_t[:, dk, :],
                                start=(dk == 0), stop=(dk == NDK - 1))
                        vt = vpool.tile([128, NL], F32R, tag="v")
                        nc.scalar.copy(vt[:], psv[:])
                        Vt[4 * g + st] = vt
                    if 0 <= i - 1 < 4:
                        emit_ss(states[i - 1])
                    if 0 <= i - 2 < 4:
                        emit_rot(states[i - 2])

                # ---- attention for each head ----
                osbs = {}
                for h in range(HLOC):
                    qr_t = qr_tiles[h]
                    kts = list(range(max(0, 4 * g - 8), 4 * g + 4))
                    oacc = psO.tile([128, SG], F32, tag="o")
                    lacc = psL.tile([128, SG], F32, tag="l")

                    pend = []

                    def emit_pv(item, first, last):
                        kt, qoff, n, p = item
                        nc.tensor.matmul(
                            oacc[:, qoff:qoff + n],
                            Vt[kt][:, h * HD:(h + 1) * HD], p[:],
                            start=first, stop=last)
                        nc.tensor.matmul(
                            lacc[:, qoff:qoff + n], ones_t[:], p[:],
                            start=first, stop=last)

                    LAG = 3
                    for i, kt in enumerate(kts):
                        qt_lo = max(4 * g, kt)
                        qt_hi = min(4 * g + 3, kt + 8)
                        qoff = 128 * (qt_lo - 4 * g)
                        n = 128 * (qt_hi - qt_lo + 1)
                        sc = psS.tile([128, n], F32, tag="score")
                        kr_t = KrT[(h, kt // 4)]
                        c = (kt % 4) * 128
                        nc.tensor.matmul(sc[:], kr_t[:, c:c + 128],
                                         qr_t[:, qoff:qoff + n],
                                         start=True, stop=True)
                        p = ppool.tile([128, n], F32R, tag="p")
                        nc.scalar.activation(p[:], sc[:], AF.Exp)
                        if kt >= 4 * g:
                            nc.vector.tensor_mul(p[:, 0:128], p[:, 0:128], md_t[:])
                        if kt + 8 <= 4 * g + 3:
                            nc.vector.tensor_mul(p[:, n - 128:n], p[:, n - 128:n],
                                                 me_t[:])
                        pend.append((kt, qoff, n, p))
                        if i >= LAG:
                            emit_pv(pend[i - LAG], first=(i - LAG == 0), last=False)
                    nk = len(kts)
                    for j in range(max(0, nk - LAG), nk):
                        emit_pv(pend[j], first=(j == 0), last=(j == nk - 1))

                    linv = lpool.tile([128, SG], F32, tag="lin")
                    nc.vector.reciprocal(linv[:], lacc[:])
                    osb = opool.tile([128, SG], F32R, tag="osb")
                    nc.vector.tensor_mul(osb[:], oacc[:], linv[:])
                    osbs[h] = osb

                # ---- Wo partial for this group ----
                for st in range(4):
                    for dg in range(4):
                        pso = psA.tile([128, 512], F32, tag="a")
                        nc.tensor.matmul(pso[:],
                                         osbs[0][:, st * 128:(st + 1) * 128],
                                         wo_t[:, 0, dg * 512:(dg + 1) * 512],
                                         start=True, stop=False)
                        nc.tensor.matmul(pso[:],
                                         osbs[1][:, st * 128:(st + 1) * 128],
                                         wo_t[:, 1, dg * 512:(dg + 1) * 512],
                                         start=False, stop=True)
                        ot = outp.tile([128, 512], F32, tag="out")
                        nc.scalar.copy(ot[:], pso[:])
                        row = s0 + st * 128
                        nc.sync.dma_start(
                            opart[row:row + 128, dg * 512:(dg + 1) * 512], ot[:])

    nc.compile()
    return nc


def _host_prep(hidden_states, cos, sin, Wq, Wk, Wv, Wo, q_scale, k_scale):
    f32 = np.float32
    hs = np.asarray(hidden_states, f32)
    cos = np.asarray(cos, f32)
    sin = np.asarray(sin, f32)
    qs = np.asarray(q_scale, f32)
    ks = np.asarray(k_scale, f32)

    xt = np.concatenate([hs[b].T for b in range(B)], axis=1)  # [D, B*S]
    xt = np.ascontiguousarray(xt, f32)

    qs_rot = np.roll(qs, -64)
    ks_rot = np.roll(ks, -64)

    def rope_tables(scale_vec, rot_vec, extra):
        c = np.concatenate([(cos[b] * scale_vec[None, :] * extra).T
                            for b in range(B)], axis=1)
        s = np.concatenate([(sin[b] * rot_vec[None, :] * extra).T
                            for b in range(B)], axis=1)
        return (np.ascontiguousarray(c, f32), np.ascontiguousarray(s, f32))

    cqa, sqa = rope_tables(qs, qs_rot, SCALE)
    cka, ska = rope_tables(ks, ks_rot, 1.0)

    ones = np.ones((128, 128), f32)
    mdiag = np.triu(np.ones((128, 128), f32))   # keep kk <= qq
    medge = np.tril(np.ones((128, 128), f32))   # keep kk >= qq
    # rotate_half as a matmul: out[p, s] = sum_k rperm[k, p] * x[k, s]
    rperm = np.zeros((128, 128), f32)
    rperm[np.arange(64) + 64, np.arange(64)] = -1.0   # out[p<64] = -x[p+64]
    rperm[np.arange(64), np.arange(64) + 64] = 1.0    # out[p>=64] = x[p-64]

    shared = {
        "xt": xt, "cq": cqa, "sq": sqa, "ck": cka, "sk": ska,
        "ones_d": ones, "mdiag_d": mdiag, "medge_d": medge, "rperm_d": rperm,
    }
    Wq = np.asarray(Wq, f32)
    Wk = np.asarray(Wk, f32)
    Wv = np.asarray(Wv, f32)
    Wo = np.asarray(Wo, f32)
    in_maps = []
    for c in range(NCORES):
        m = dict(shared)
        m["wq"] = np.ascontiguousarray(Wq[:, c * NL:(c + 1) * NL])
        m["wk"] = np.ascontiguousarray(Wk[:, c * NL:(c + 1) * NL])
        m["wv"] = np.ascontiguousarray(Wv[:, c * NL:(c + 1) * NL])
        m["wo"] = np.ascontiguousarray(Wo[c * NL:(c + 1) * NL, :])
        in_maps.append(m)
    return in_maps


def get_nc(repeat=1):
    key = ("nc", repeat)
    if key not in _CACHE:
        _CACHE[key] = _build(repeat=repeat)
    return _CACHE[key]


def kernel(hidden_states, cos, sin, Wq, Wk, Wv, Wo, q_scale, k_scale):
    nc = get_nc()
    in_maps = _host_prep(hidden_states, cos, sin, Wq, Wk, Wv, Wo,
                         q_scale, k_scale)
    res = bass_utils.run_bass_kernel_spmd(nc, in_maps,
                                          core_ids=list(range(NCORES)))
    acc = np.zeros((B * S, D), np.float64)
    for r in res.results:
        acc += r["opart"].astype(np.float64)
    return np.ascontiguousarray(
        acc.reshape(B, S, D).astype(np.float32))


# revision 16
# speedup vs baseline: 21.5133x; 1.2795x over previous
"""Trainium2 Bass kernel for nn_MultiHeadAttention_8400956031164.

Full attention block: QKV proj + per-head RMSNorm + RoPE + sliding-window
causal attention (WIN=1024) + output proj.

Sharding: tensor-parallel over heads across 8 cores (2 heads/core), both
batches looped per core. Host sums the 8 partial Wo outputs.

Device-side layout strategy (per core):
  - X^T [D, S] streamed per 512-token group; Q,K produced TRANSPOSED
    [hd=128, s] per head directly from projection (lhsT = W slice).
  - RMSNorm in transposed layout: sum(q^2) over hd via all-ones matmul
    (result broadcast across partitions in PSUM), sqrt on ACT, reciprocal
    on DVE, multiply. q_scale/k_scale/softmax SCALE folded into RoPE
    cos/sin tables on host.
  - RoPE in transposed layout: rotate_half = partition-half swap; sign
    folded into host-precomputed sin tables.
  - Scores computed transposed S^T[k, q] (k on partitions) per 128x(<=512)
    block over the sliding window; exp on ACT (PSUM->SBUF, f32r out);
    causal/window triangle masks applied by DVE mask-multiply.
  - Softmax denominator via all-ones matmul accumulation (broadcast sums);
    PV accumulates V^T @ P^T = out^T [hd, q] in PSUM with variable-N
    has_written semantics; normalization folded as out^T * (1/L).
  - Wo: lhsT = normalized out^T slices, accumulate 2 head-chunks, ACT
    copy PSUM->SBUF, DMA out.
  All matmuls in float32r (TF32-class, ~1.5e-4 relerr, full PE rate).
"""

import numpy as np
from contextlib import ExitStack

import concourse.bass as bass
import concourse.tile as tile
import concourse.mybir as mybir
from concourse import bacc, bass_utils

B, S, D, H, HD, WIN = 2, 2048, 2048, 16, 128, 1024
EPS = 1e-6
SCALE = HD ** -0.5
NCORES = 8
HLOC = H // NCORES          # heads per core = 2
NL = HLOC * HD              # local head dims = 256
SG = 512                    # token group size
G = S // SG                 # groups per batch = 4
NDK = D // 128              # contraction chunks = 16

F32 = mybir.dt.float32
F32R = mybir.dt.float32r
AF = mybir.ActivationFunctionType

_CACHE = {}


def _build(repeat=1):
    nc = bacc.Bacc(trn_type="TRN2", target_bir_lowering=False, debug=False)

    def din(name, shape, dt):
        return nc.dram_tensor(name, shape, dt, kind="ExternalInput").ap()

    xt = din("xt", [D, B * S], F32R)
    wq = din("wq", [D, NL], F32R)
    wk = din("wk", [D, NL], F32R)
    wv = din("wv", [D, NL], F32R)
    wo = din("wo", [NL, D], F32R)
    cq = din("cq", [HD, B * S], F32)   # cos * q_scale * SCALE, transposed
    sq = din("sq", [HD, B * S], F32)   # sin * roll(q_scale) * SCALE, lower half negated
    ck = din("ck", [HD, B * S], F32)
    sk = din("sk", [HD, B * S], F32)
    ones_d = din("ones_d", [128, 128], F32R)
    rperm_d = din("rperm_d", [128, 128], F32R)  # rotate_half as matmul, signs included
    mdiag_d = din("mdiag_d", [128, 128], F32)   # keep kk <= qq
    medge_d = din("medge_d", [128, 128], F32)   # keep kk >= qq
    opart = nc.dram_tensor("opart", [B * S, D], F32, kind="ExternalOutput").ap()

    xtr = xt.rearrange("(a p) s -> p a s", p=128)     # [128, 16, B*S]
    wqr = wq.rearrange("(a p) n -> p a n", p=128)     # [128, 16, NL]
    wkr = wk.rearrange("(a p) n -> p a n", p=128)
    wvr = wv.rearrange("(a p) n -> p a n", p=128)
    wor = wo.rearrange("(c p) d -> p c d", p=128)     # [128, 2, D]

    with tile.TileContext(nc) as tc, ExitStack() as ctx:
        const = ctx.enter_context(tc.tile_pool(name="const", bufs=1))
        wpool = ctx.enter_context(tc.tile_pool(name="w", bufs=1))
        xpool = ctx.enter_context(tc.tile_pool(name="x", bufs=2))
        cspool = ctx.enter_context(tc.tile_pool(name="cs", bufs=2))
        qpool = ctx.enter_context(tc.tile_pool(name="qr", bufs=3))
        kpool = ctx.enter_context(tc.tile_pool(name="kr", bufs=10))
        vpool = ctx.enter_context(tc.tile_pool(name="v", bufs=18))
        rpool = ctx.enter_context(tc.tile_pool(name="rms", bufs=2))
        ppool = ctx.enter_context(tc.tile_pool(name="p", bufs=6))
        lpool = ctx.enter_context(tc.tile_pool(name="lin", bufs=2))
        opool = ctx.enter_context(tc.tile_pool(name="osb", bufs=3))
        outp = ctx.enter_context(tc.tile_pool(name="out", bufs=3))
        psA = ctx.enter_context(tc.tile_pool(name="psA", bufs=3, space="PSUM"))
        psS = ctx.enter_context(tc.tile_pool(name="psS", bufs=3, space="PSUM"))
        psO = ctx.enter_context(tc.tile_pool(name="psO", bufs=1, space="PSUM"))
        psL = ctx.enter_context(tc.tile_pool(name="psL", bufs=1, space="PSUM"))

        ones_t = const.tile([128, 128], F32R, tag="ones")
        nc.sync.dma_start(ones_t[:], ones_d)
        rperm_t = const.tile([128, 128], F32R, tag="rperm")
        nc.sync.dma_start(rperm_t[:], rperm_d)
        md_t = const.tile([128, 128], F32, tag="md")
        nc.sync.dma_start(md_t[:], mdiag_d)
        me_t = const.tile([128, 128], F32, tag="me")
        nc.sync.dma_start(me_t[:], medge_d)
        eps_t = const.tile([128, 1], F32, tag="eps")
        nc.vector.memset(eps_t[:], EPS)

        wq_t = wpool.tile([128, NDK, NL], F32R, tag="wq")
        nc.sync.dma_start(wq_t[:], wqr)
        wk_t = wpool.tile([128, NDK, NL], F32R, tag="wk")
        nc.sync.dma_start(wk_t[:], wkr)
        wv_t = wpool.tile([128, NDK, NL], F32R, tag="wv")
        nc.sync.dma_start(wv_t[:], wvr)
        wo_t = wpool.tile([128, HLOC, D], F32R, tag="wo")
        nc.sync.dma_start(wo_t[:], wor)

        for b in [b_ for _ in range(repeat) for b_ in range(B)]:
            KrT = {}   # (h, g) -> [128, SG] f32r tile
            Vt = {}    # s-tile index -> [128, NL] f32r tile
            for g in range(G):
                s0 = b * S + g * SG

                # ---- input streams for this group ----
                xh0 = xpool.tile([128, 8, SG], F32R, tag="xt")
                nc.sync.dma_start(xh0[:], xtr[:, 0:8, s0:s0 + SG])
                xh1 = xpool.tile([128, 8, SG], F32R, tag="xt")
                nc.sync.dma_start(xh1[:], xtr[:, 8:16, s0:s0 + SG])

                def xs(dk):
                    t = xh0 if dk < 8 else xh1
                    return t[:, dk % 8, :]

                cqt = cspool.tile([128, SG], F32, tag="cq")
                nc.sync.dma_start(cqt[:], cq[:, s0:s0 + SG])
                sqt = cspool.tile([128, SG], F32, tag="sq")
                nc.sync.dma_start(sqt[:], sq[:, s0:s0 + SG])
                ckt = cspool.tile([128, SG], F32, tag="ck")
                nc.sync.dma_start(ckt[:], ck[:, s0:s0 + SG])
                skt = cspool.tile([128, SG], F32, tag="sk")
                nc.sync.dma_start(skt[:], sk[:, s0:s0 + SG])

                # ---- Q/K transposed projections + RMSNorm + RoPE ----
                # PE pipelining: after each projection chain, emit the
                # previous chain's sum-of-squares matmul and the chain
                # before that's rotate matmul, so PE never waits on ACT/DVE.
                qr_tiles = {}

                states = []

                def emit_ss(stt):
                    ssps = psS.tile([128, SG], F32, tag="score")
                    nc.tensor.matmul(ssps[:], ones_t[:], stt["qsq"][:],
                                     start=True, stop=True)
                    # 1/sqrt(v) = exp(-0.5*ln(v)) keeps every ACT func in the
                    # natural_log_exp_and_others table set (no table thrash).
                    rstd = rpool.tile([128, SG], F32, tag="rstd")
                    nc.scalar.activation(rstd[:], ssps[:], AF.Ln,
                                         bias=eps_t[:, 0:1], scale=1.0 / HD)
                    nc.scalar.activation(rstd[:], rstd[:], AF.Exp, scale=-0.5)
                    qn = rpool.tile([128, SG], F32R, tag="qn")
                    nc.vector.tensor_mul(qn[:], stt["ps"][:], rstd[:])
                    t1 = rpool.tile([128, SG], F32, tag="t1")
                    cost = cqt if stt["t"] == "q" else ckt
                    nc.vector.tensor_mul(t1[:], qn[:], cost[:])
                    stt["qn"] = qn
                    stt["t1"] = t1

                def emit_rot(stt):
                    rot_ps = psS.tile([128, SG], F32, tag="score")
                    nc.tensor.matmul(rot_ps[:], rperm_t[:], stt["qn"][:],
                                     start=True, stop=True)
                    sint = sqt if stt["t"] == "q" else skt
                    dst = stt["dst"]
                    nc.vector.tensor_mul(dst[:], rot_ps[:], sint[:])
                    nc.vector.tensor_add(dst[:], dst[:], stt["t1"][:])

                chains = [("q", 0), ("k", 0), ("q", 1), ("k", 1),
                          ("v", 0), ("v", 1), ("v", 2), ("v", 3)]
                for i, (t, h) in enumerate(chains):
                    if t in ("q", "k"):
                        w_t = wq_t if t == "q" else wk_t
                        ps = psA.tile([128, SG], F32, tag="a")
                        for dk in range(NDK):
                            nc.tensor.matmul(
                                ps[:], w_t[:, dk, h * HD:(h + 1) * HD], xs(dk),
                                start=(dk == 0), stop=(dk == NDK - 1))
                        qsq = rpool.tile([128, SG], F32R, tag="qsq")
                        nc.scalar.activation(qsq[:], ps[:], AF.Square)
                        if t == "q":
                            dst = qpool.tile([128, SG], F32R, tag="qr")
                            qr_tiles[h] = dst
                        else:
                            dst = kpool.tile([128, SG], F32R, tag="kr")
                            KrT[(h, g)] = dst
                        states.append({"ps": ps, "qsq": qsq, "t": t, "dst": dst})
                    else:
                        st = h
                        psv = psA.tile([128, NL], F32, tag="a")
                        for dk in range(NDK):
                            nc.tensor.matmul(
                                psv[:], xs(dk)[:, st * 128:(st + 1) * 128],
                                wv_t[:, dk, :],
                                start=(dk == 0), stop=(dk == NDK - 1))
                        vt = vpool.tile([128, NL], F32R, tag="v")
                        nc.scalar.copy(vt[:], psv[:])
                        Vt[4 * g + st] = vt
                    if 0 <= i - 1 < 4:
                        emit_ss(states[i - 1])
                    if 0 <= i - 2 < 4:
                        emit_rot(states[i - 2])

                # ---- attention for each head ----
                osbs = {}
                for h in range(HLOC):
                    qr_t = qr_tiles[h]
                    kts = list(range(max(0, 4 * g - 8), 4 * g + 4))
                    oacc = psO.tile([128, SG], F32, tag="o")
                    lacc = psL.tile([128, SG], F32, tag="l")

                    pend = []

                    def emit_pv(item, first, last):
                        kt, qoff, n, p = item
                        nc.tensor.matmul(
                            oacc[:, qoff:qoff + n],
                            Vt[kt][:, h * HD:(h + 1) * HD], p[:],
                            start=first, stop=last)
                        nc.tensor.matmul(
                            lacc[:, qoff:qoff + n], ones_t[:], p[:],
                            start=first, stop=last)

                    LAG = 3
                    for i, kt in enumerate(kts):
                        qt_lo = max(4 * g, kt)
                        qt_hi = min(4 * g + 3, kt + 8)
                        qoff = 128 * (qt_lo - 4 * g)
                        n = 128 * (qt_hi - qt_lo + 1)
                        sc = psS.tile([128, n], F32, tag="score")
                        kr_t = KrT[(h, kt // 4)]
                        c = (kt % 4) * 128
                        nc.tensor.matmul(sc[:], kr_t[:, c:c + 128],
                                         qr_t[:, qoff:qoff + n],
                                         start=True, stop=True)
                        p = ppool.tile([128, n], F32R, tag="p")
                        nc.scalar.activation(p[:], sc[:], AF.Exp)
                        if kt >= 4 * g:
                            nc.vector.tensor_mul(p[:, 0:128], p[:, 0:128], md_t[:])
                        if kt + 8 <= 4 * g + 3:
                            nc.vector.tensor_mul(p[:, n - 128:n], p[:, n - 128:n],
                                                 me_t[:])
                        pend.append((kt, qoff, n, p))
                        if i >= LAG:
                            emit_pv(pend[i - LAG], first=(i - LAG == 0), last=False)
                    nk = len(kts)
                    for j in range(max(0, nk - LAG), nk):
                        emit_pv(pend[j], first=(j == 0), last=(j == nk - 1))

                    linv = lpool.tile([128, SG], F32, tag="lin")
                    nc.vector.reciprocal(linv[:], lacc[:])
                    osb = opool.tile([128, SG], F32R, tag="osb")
                    nc.vector.tensor_mul(osb[:], oacc[:], linv[:])
                    osbs[h] = osb

                # ---- Wo partial for this group ----
                for st in range(4):
                    for dg in range(4):
                        pso = psA.tile([128, 512], F32, tag="a")
                        nc.tensor.matmul(pso[:],
                                         osbs[0][:, st * 128:(st + 1) * 128],
                                         wo_t[:, 0, dg * 512:(dg + 1) * 512],
                                         start=True, stop=False)
                        nc.tensor.matmul(pso[:],
                                         osbs[1][:, st * 128:(st + 1) * 128],
                                         wo_t[:, 1, dg * 512:(dg + 1) * 512],
                                         start=False, stop=True)
                        ot = outp.tile([128, 512], F32, tag="out")
                        nc.scalar.copy(ot[:], pso[:])
                        row = s0 + st * 128
                        nc.sync.dma_start(
                            opart[row:row + 128, dg * 512:(dg + 1) * 512], ot[:])

    nc.compile()
    return nc


def _host_prep(hidden_states, cos, sin, Wq, Wk, Wv, Wo, q_scale, k_scale):
    f32 = np.float32
    hs = np.asarray(hidden_states, f32)
    cos = np.asarray(cos, f32)
    sin = np.asarray(sin, f32)
    qs = np.asarray(q_scale, f32)
    ks = np.asarray(k_scale, f32)

    xt = np.concatenate([hs[b].T for b in range(B)], axis=1)  # [D, B*S]
    xt = np.ascontiguousarray(xt, f32)

    qs_rot = np.roll(qs, -64)
    ks_rot = np.roll(ks, -64)

    def rope_tables(scale_vec, rot_vec, extra):
        c = np.concatenate([(cos[b] * scale_vec[None, :] * extra).T
                            for b in range(B)], axis=1)
        s = np.concatenate([(sin[b] * rot_vec[None, :] * extra).T
                            for b in range(B)], axis=1)
        return (np.ascontiguousarray(c, f32), np.ascontiguousarray(s, f32))

    cqa, sqa = rope_tables(qs, qs_rot, SCALE)
    cka, ska = rope_tables(ks, ks_rot, 1.0)

    ones = np.ones((128, 128), f32)
    mdiag = np.triu(np.ones((128, 128), f32))   # keep kk <= qq
    medge = np.tril(np.ones((128, 128), f32))   # keep kk >= qq
    # rotate_half as a matmul: out[p, s] = sum_k rperm[k, p] * x[k, s]
    rperm = np.zeros((128, 128), f32)
    rperm[np.arange(64) + 64, np.arange(64)] = -1.0   # out[p<64] = -x[p+64]
    rperm[np.arange(64), np.arange(64) + 64] = 1.0    # out[p>=64] = x[p-64]

    shared = {
        "xt": xt, "cq": cqa, "sq": sqa, "ck": cka, "sk": ska,
        "ones_d": ones, "mdiag_d": mdiag, "medge_d": medge, "rperm_d": rperm,
    }
    Wq = np.asarray(Wq, f32)
    Wk = np.asarray(Wk, f32)
    Wv = np.asarray(Wv, f32)
    Wo = np.asarray(Wo, f32)
    in_maps = []
    for c in range(NCORES):
        m = dict(shared)
        m["wq"] = np.ascontiguousarray(Wq[:, c * NL:(c + 1) * NL])
        m["wk"] = np.ascontiguousarray(Wk[:, c * NL:(c + 1) * NL])
        m["wv"] = np.ascontiguousarray(Wv[:, c * NL:(c + 1) * NL])
        m["wo"] = np.ascontiguousarray(Wo[c * NL:(c + 1) * NL, :])
        in_maps.append(m)
    return in_maps


def get_nc(repeat=1):
    key = ("nc", repeat)
    if key not in _CACHE:
        _CACHE[key] = _build(repeat=repeat)
    return _CACHE[key]


def kernel(hidden_states, cos, sin, Wq, Wk, Wv, Wo, q_scale, k_scale):
    nc = get_nc()
    in_maps = _host_prep(hidden_states, cos, sin, Wq, Wk, Wv, Wo,
                         q_scale, k_scale)
    res = bass_utils.run_bass_kernel_spmd(nc, in_maps,
                                          core_ids=list(range(NCORES)))
    acc = np.zeros((B * S, D), np.float64)
    for r in res.results:
        acc += r["opart"].astype(np.float64)
    return np.ascontiguousarray(
        acc.reshape(B, S, D).astype(np.float32))
